# revision 7
# baseline (speedup 1.0000x reference)
"""Trainium2 Bass kernel for nn_CausalSelfAttention_74268574482879.

The reference module's attention scores are overwritten by the causal mask
(q/k are discarded), so softmax weights are uniform over positions <= t:
    y = cummean_T(x) @ W_v @ W_p,   W_v = w_attn[:, 1024:1536]

Distribution: the 4096 rows of (B*T) are split into 8 chunks of 512 rows,
one per NeuronCore.  The cross-chunk dependency (column-sum of all earlier
rows in the same batch element) is folded into row 0 of each chunk on the
host while slicing the shards, so the device sees a plain per-chunk prefix
sum.

Per-core dataflow, all-bf16 operands with fp32 PSUM accumulation:
  warmup:  a few dummy matmuls on memset scratch ramp the PE DVFS clock
           to 2.4 GHz while the first x piece is still in flight.
  stage A: per input-feature block i, the full 512-row cumsum lands in one
           PSUM bank via 4 accumulating matmuls against G = [triu | ones]
           (the ones columns add each row-tile's colsum into all later
           tiles) -- no vector-engine carries at all.  k-outer order so
           round k only needs the k-th x DMA piece.
  stage B: M1^T = (A @ Wv)^T    (4 PSUM banks, 16 matmuls)
  stage C: Y = M1 @ Wp          (4 PSUM banks, 16 matmuls), evicted with
           the fused 1/(t+1) per-row scale, bf16, DMA'd out per row-tile.

PSUM->SBUF evictions are split in half across DVE and Act so the stage
boundaries stall the PE as little as possible.  All HBM tensors are
host-packed to exactly match the SBUF tile layouts so every DMA is a plain
contiguous (128 x N) transfer.
"""

import numpy as np
import ml_dtypes

import concourse.bass as bass
import concourse.bacc as bacc
import concourse.mybir as mybir
import concourse.tile as tile
from concourse import bass_utils

N_CORES = 8
B, T, C = 2, 2048, 512
CHUNK = 512               # rows of flattened (B*T) per core
P = 128
NT = CHUNK // P           # 4 row-tiles per chunk
NI = C // P               # 4 col-tiles of the 512 feature dim
F32 = mybir.dt.float32
BF16 = mybir.dt.bfloat16
BF16NP = ml_dtypes.bfloat16

XGW = NT * C + C          # xg tensor width: [G | X]
N_WARM = 5                # dummy matmuls to ramp the PE clock

TRACE = [False]
LAST_RESULT = [None]
_STATE = {}


def _build_nc():
    nc = bacc.Bacc(
        "TRN2", target_bir_lowering=False, debug=False, num_devices=N_CORES
    )
    xg_d = nc.dram_tensor("xg", (P, XGW), BF16, kind="ExternalInput")
    wv_d = nc.dram_tensor("wv", (P, NI * C), BF16, kind="ExternalInput")
    wp_d = nc.dram_tensor("wp", (P, NI * C), BF16, kind="ExternalInput")
    sc_d = nc.dram_tensor("sc", (P, NT), F32, kind="ExternalInput")
    y_d = nc.dram_tensor("y", (P, NT * C), BF16, kind="ExternalOutput")

    xg_ap, wv_ap, wp_ap = xg_d.ap(), wv_d.ap(), wp_d.ap()
    sc_ap, y_ap = sc_d.ap(), y_d.ap()

    with tile.TileContext(nc) as tc:
        with (
            tc.tile_pool(name="io", bufs=1) as io,
            tc.tile_pool(name="ps", bufs=1, space="PSUM") as ps,
        ):
            xg = io.tile([P, XGW], BF16, name="xg")
            wv = io.tile([P, NI * C], BF16, name="wv")
            wp = io.tile([P, NI * C], BF16, name="wp")
            sc = io.tile([P, NT], F32, name="sc")
            A_sb = io.tile([P, NI, CHUNK], BF16, name="A")
            M1_sb = io.tile([P, NI, CHUNK], BF16, name="M1")
            y_sb = io.tile([P, NT * C], BF16, name="y")
            scratch = io.tile([P, C], BF16, name="scratch")

            # ---- input DMAs: contiguous pieces, pipelined across queues.
            # sync+scalar are HWDGE engines (cheap issue) and carry the
            # x pieces that gate compute; weights ride the Pool SWDGE
            # queue, which is free this early.
            nc.sync.dma_start(xg[:, 0:C], xg_ap[:, 0:C])            # G
            nc.scalar.dma_start(xg[:, C : 2 * C], xg_ap[:, C : 2 * C])  # Xk0
            nc.sync.dma_start(xg[:, 2 * C : 3 * C], xg_ap[:, 2 * C : 3 * C])
            nc.scalar.dma_start(xg[:, 3 * C : XGW], xg_ap[:, 3 * C : XGW])
            nc.gpsimd.dma_start(wv[:], wv_ap[:, :])
            nc.gpsimd.dma_start(wp[:], wp_ap[:, :])
            nc.gpsimd.dma_start(sc[:], sc_ap[:, :])

            # ---- PE warmup: ramp DVFS while the x DMA is in flight ----
            nc.gpsimd.memset(scratch[:], 0)
            warm = ps.tile([P, C], F32, name="warm", tag="ay", bufs=4)
            for w in range(N_WARM):
                nc.tensor.matmul(
                    warm[:],
                    scratch[:, 0:P],
                    scratch[:],
                    start=(w == 0),
                    stop=(w == N_WARM - 1),
                )

            # ---- stage A: cumsum via accumulating matmuls ----
            # pa[:, k*P:] += Xk_i^T @ [US | 1...]; the ones block adds tile
            # k's colsum into every later row-tile, PSUM does the carry.
            pas = [
                ps.tile([P, CHUNK], F32, name=f"pA{i}", tag="ay", bufs=4)
                for i in range(NI)
            ]
            for k in range(NT):
                for i in range(NI):
                    xoff = C + k * C + i * P
                    nc.tensor.matmul(
                        pas[i][:, k * P : CHUNK],
                        xg[:, xoff : xoff + P],
                        xg[:, 0 : (NT - k) * P],
                        start=(k == 0),
                        stop=(k == NT - 1),
                    )
                    if k == NT - 1:
                        h = CHUNK // 2
                        nc.vector.tensor_copy(
                            A_sb[:, i, 0:h], pas[i][:, 0:h]
                        )
                        nc.scalar.copy(
                            A_sb[:, i, h:CHUNK], pas[i][:, h:CHUNK]
                        )

            # ---- stage B: M1^T = (A @ Wv)^T ----
            for jj in range(NI):
                pm = ps.tile([P, CHUNK], F32, name=f"pM{jj}", tag="m", bufs=4)
                for i in range(NI):
                    nc.tensor.matmul(
                        pm[:],
                        wv[:, i * C + jj * P : i * C + (jj + 1) * P],
                        A_sb[:, i, :],
                        start=(i == 0),
                        stop=(i == NI - 1),
                    )
                h = CHUNK // 2
                nc.vector.tensor_copy(M1_sb[:, jj, 0:h], pm[:, 0:h])
                nc.scalar.copy(M1_sb[:, jj, h:CHUNK], pm[:, h:CHUNK])

            # ---- stage C: Y = M1 @ Wp, scaled eviction, DMA out ----
            for t in range(NT):
                py = ps.tile([P, C], F32, name=f"pY{t}", tag="ay", bufs=4)
                for jj in range(NI):
                    nc.tensor.matmul(
                        py[:],
                        M1_sb[:, jj, t * P : (t + 1) * P],
                        wp[:, jj * C : (jj + 1) * C],
                        start=(jj == 0),
                        stop=(jj == NI - 1),
                    )
                h = C // 2
                o = t * C
                nc.vector.tensor_scalar_mul(
                    y_sb[:, o : o + h], py[:, 0:h], sc[:, t : t + 1]
                )
                nc.scalar.mul(
                    y_sb[:, o + h : o + C], py[:, h:C], sc[:, t : t + 1]
                )
                nc.sync.dma_start(
                    y_ap[:, o : o + C], y_sb[:, o : o + C]
                )

    nc.compile()
    return nc


def _get_nc():
    if "nc" not in _STATE:
        _STATE["nc"] = _build_nc()
    return _STATE["nc"]


def _prepare_in_maps(x, w_attn, w_proj):
    x = np.asarray(x, dtype=np.float32)
    w_attn = np.asarray(w_attn, dtype=np.float32)
    w_proj = np.asarray(w_proj, dtype=np.float32)

    wv = w_attn[:, 2 * C : 3 * C]
    WV = np.ascontiguousarray(
        wv.reshape(NI, P, C).transpose(1, 0, 2).reshape(P, NI * C)
    ).astype(BF16NP)
    WP = np.ascontiguousarray(
        w_proj.reshape(NI, P, C).transpose(1, 0, 2).reshape(P, NI * C)
    ).astype(BF16NP)
    US = np.triu(np.ones((P, P), np.float32))  # US[s, n] = 1 for s <= n
    G = np.concatenate([US, np.ones((P, (NT - 1) * P), np.float32)], axis=1)

    in_maps = []
    for core in range(N_CORES):
        b, tc = divmod(core, T // CHUNK)
        goff = tc * CHUNK
        xc = x[b, goff : goff + CHUNK].copy()
        if goff:
            # fold the cross-chunk prefix into row 0: the per-chunk cumsum
            # then reproduces halo + local prefix at every row
            xc[0] += x[b, :goff].sum(axis=0, dtype=np.float32)
        X = xc.reshape(NT, P, NI, P).transpose(1, 0, 2, 3).reshape(P, NT * C)
        XG = np.concatenate([G, X], axis=1).astype(BF16NP)
        scale = (1.0 / (goff + np.arange(1, CHUNK + 1))).astype(np.float32)
        SC = np.ascontiguousarray(scale.reshape(NT, P).T)
        in_maps.append({"xg": XG, "wv": WV, "wp": WP, "sc": SC})
    return in_maps


def kernel(x, w_attn, w_proj):
    nc = _get_nc()
    in_maps = _prepare_in_maps(x, w_attn, w_proj)
    res = bass_utils.run_bass_kernel_spmd(
        nc, in_maps, core_ids=list(range(N_CORES)), trace=TRACE[0]
    )
    LAST_RESULT[0] = res
    y = np.empty((B, T, C), np.float32)
    for core in range(N_CORES):
        b, tc = divmod(core, T // CHUNK)
        Y = np.asarray(res.results[core]["y"]).astype(np.float32)
        y[b, tc * CHUNK : (tc + 1) * CHUNK, :] = (
            Y.reshape(P, NT, C).transpose(1, 0, 2).reshape(CHUNK, C)
        )
    return y
